# revision 2
# baseline (speedup 1.0000x reference)
"""Trainium2 Bass kernel for nn_DctAtt (B=32, D=1024, N=4096, K=5).

The reference collapses to att[b,d] = x[b,d,:] . w  (w = C @ dw_w precomputed
on host), followed by tiny [32,1024] BN/GELU/softmax work done on host.
The device kernel streams x (512 MiB, data-parallel over B across 8 cores,
64 MiB/core) through fused DVE tensor_tensor_reduce dot products.

Trace findings this design is built on (see ntff profiles):
  * Every HWDGE dma_start stripes uniformly over the 16 SDMA engines by
    destination SBUF partition (engine k serves a fixed set of 8 partitions).
    Engines 0-14 sustain ~26.5 GB/s each (aggregate ~428 GB/s = SBUF AXI
    fabric ceiling), but engine 15 intermittently runs at ~22 GB/s
    (known-slow engine), turning uniform striping into a 181 vs 221 us
    bimodal kernel: everyone else finishes and idles while engine 15 drains.
  * Fix: "slim" units skip engine 15's partitions {92-95, 124-127} so
    engine 15 carries only ~75% of the per-engine load and is never the
    critical path, in either its fast or slow mode.
  * The old [128, 4096] w replica cost a 2 MiB HBM read serialized ahead of
    the x stream (~6 us). Now w is loaded as one 16 KiB row and broadcast
    across partitions on-chip with a K=1 TensorE matmul (ones[1,128]^T @
    w[1,512]) through PSUM -- zero extra DMA-engine work.
  * Work granularity is a half row (2048 cols, 8 KiB per partition) so the
    slim/full mix can hit the engine-load ratio closely, and the last unit
    is further split into 1024-col quarters so only ~1.1 us of DVE work
    remains after the final DMA byte.

Unit plan (per core, 4096 rows x 4096 cols = 8192 half-rows):
  65 units of [<=128 partitions, 2048 cols]; 16 slim units (120 rows on
  partitions 0-91 & 96-123) interleaved every 4th unit, 49 full units.
  49*128 + 16*120 = 8192.  Half h=0 -> first 32 full units, h=1 -> rest.
  y_sb[p, u] = dot(half-row, w-half); host sums the two halves per row.
"""

import math

import numpy as np

import concourse.bacc as bacc
import concourse.mybir as mybir
import concourse.tile as tile
from concourse import bass_utils

# Problem constants (hardcoded: the grading harness ships only this file).
B, D, N = 32, 1024, 4096
K = 5
BN_EPS = 1e-5
N_CORES = 8
P = 128
HALF = N // 2  # 2048
ROWS_PER_CORE = (B // N_CORES) * D  # 4096

import os as _os

# In-flight [128, 2048] tiles (8 KiB/partition each).
XP_BUFS = int(_os.environ.get("DCT_BUFS", "12"))
# 1: de-weight SDMA engine 15 via slim units; 0: uniform 64 full units.
SLIM = int(_os.environ.get("DCT_SLIM", "1"))
# 1: split the last unit into two 1024-col chunks to shrink the DVE tail.
TAILSPLIT = int(_os.environ.get("DCT_TAILSPLIT", "1"))
# 1: broadcast w on-chip via PE (16 KiB HBM); 0: read [128, N] replica (2 MiB).
PEW = int(_os.environ.get("DCT_PEW", "1"))

# SDMA engine 15 serves these SBUF partitions (port swizzle: odd engine
# 2j+1 -> {64+4j..64+4j+3, 96+4j..96+4j+3}, j=7).
E15_SPANS = ((0, 92), (96, 28))  # (partition0, nrows) spans avoiding eng 15
FULL_SPANS = ((0, 128),)


def _unit_plan():
    """Static unit list shared by the device build and the host gather.

    Returns (units, n_ycols); units[i] = dict(h, row0, spans, cols, ycols)
    where ycols lists the y_sb columns holding this unit's (partial) dots.
    """
    if SLIM:
        n_units = 65
        slim = [(u % 4 == 3) and u < 64 for u in range(n_units)]
    else:
        n_units = 64
        slim = [False] * n_units
    full_seen = 0
    units = []
    cur = [0, 0]
    for u in range(n_units):
        spans = E15_SPANS if slim[u] else FULL_SPANS
        cap = sum(c for _, c in spans)
        if not slim[u]:
            h = 0 if full_seen < 32 else 1
            full_seen += 1
        else:
            h = 1
        units.append({"h": h, "row0": cur[h], "spans": spans, "u": u})
        cur[h] += cap
    assert cur[0] == ROWS_PER_CORE and cur[1] == ROWS_PER_CORE, cur
    n_ycols = n_units
    for unit in units:
        unit["ycols"] = [unit["u"]]
    if TAILSPLIT:
        units[-1]["ycols"] = [units[-1]["u"], n_units]
        n_ycols += 1
    return units, n_ycols


_compiled_nc = None


def _build():
    """Build + compile the per-core Bass program (cached per process)."""
    global _compiled_nc
    if _compiled_nc is not None:
        return _compiled_nc

    units, n_ycols = _unit_plan()
    nc = bacc.Bacc(
        "TRN2",
        target_bir_lowering=False,
        debug=False,
        enable_asserts=False,
        num_devices=N_CORES,
    )
    f32 = mybir.dt.float32
    x_sh = nc.dram_tensor("x_sh", [ROWS_PER_CORE, N], f32, kind="ExternalInput").ap()
    if PEW:
        w_in = nc.dram_tensor("w_row", [1, N], f32, kind="ExternalInput").ap()
    else:
        w_in = nc.dram_tensor("w_rep", [P, N], f32, kind="ExternalInput").ap()
    y_out = nc.dram_tensor("y_out", [P, n_ycols], f32, kind="ExternalOutput").ap()

    with tile.TileContext(nc) as tc:
        with (
            tc.tile_pool(name="wp", bufs=1) as wp,
            tc.tile_pool(name="xp", bufs=XP_BUFS) as xp,
            tc.tile_pool(name="sp", bufs=1) as sp,
            tc.tile_pool(name="yp", bufs=1) as yp,
        ):
            w_sb = wp.tile([P, N], f32)
            if PEW:
                # w: one 16 KiB HBM read on the Scalar HWDGE ring, then
                # partition-broadcast through the PE (ones[1,128].T @ w[1,c])
                # and ACT copies PSUM -> SBUF. Zero SDMA cost vs the 2 MiB
                # replica read that used to head the x stream.
                w_row = wp.tile([1, N], f32)
                ones = wp.tile([1, P], f32)
                nc.scalar.dma_start(out=w_row, in_=w_in)
                nc.vector.memset(ones, 1.0)
                cw = 512  # one PSUM bank of f32
                with tc.tile_pool(name="ps", bufs=2, space="PSUM") as psp:
                    for c in range(N // cw):
                        ps = psp.tile([P, cw], f32)
                        nc.tensor.matmul(
                            ps,
                            ones,
                            w_row[:, c * cw : (c + 1) * cw],
                            start=True,
                            stop=True,
                        )
                        nc.scalar.copy(out=w_sb[:, c * cw : (c + 1) * cw], in_=ps)
            else:
                nc.scalar.dma_start(out=w_sb, in_=w_in)
            y_sb = yp.tile([P, n_ycols], f32)
            # Stride-0 free dim: the fused op's elementwise product is not
            # materialised (every element lands on the same column).
            dummy = sp.tile([P, 1], f32)
            for unit in units:
                h, row0, spans = unit["h"], unit["row0"], unit["spans"]
                c0 = h * HALF
                ycols = unit["ycols"]
                nch = len(ycols)  # 1, or 2 for the tail-split last unit
                cw = HALF // nch
                xt = xp.tile([P, HALF], f32)
                for ci, yc in enumerate(ycols):
                    a, b = c0 + ci * cw, c0 + (ci + 1) * cw
                    r = row0
                    for p0, cnt in spans:
                        nc.sync.dma_start(
                            out=xt[p0 : p0 + cnt, ci * cw : (ci + 1) * cw],
                            in_=x_sh[r : r + cnt, a:b],
                        )
                        r += cnt
                    # accum = sum(x * w) per partition = half-row dot.
                    # Slim units leave engine-15's partitions untouched;
                    # their stale contents only pollute y rows the host
                    # never reads.
                    nc.vector.affine_mul_reduce(
                        out=dummy.broadcast_to((P, cw)),
                        accum_out=y_sb[:, yc : yc + 1],
                        in0=xt[:, ci * cw : (ci + 1) * cw],
                        in1=w_sb[:, a:b],
                        scale=1.0,
                        bias=0.0,
                    )
            nc.sync.dma_start(out=y_out, in_=y_sb)

    nc.compile()
    _compiled_nc = nc
    return nc


def _dct_weight(dw_w):
    """w = C @ dw_w in float64, where C is the [N, K] ortho DCT-II basis."""
    n = np.arange(N, dtype=np.float64)
    k = np.arange(K, dtype=np.float64)
    C = np.cos(np.pi * (2.0 * n[:, None] + 1.0) * k[None, :] / (2.0 * N))
    C *= math.sqrt(2.0 / N)
    C[:, 0] *= 1.0 / math.sqrt(2.0)
    return (C @ np.asarray(dw_w, dtype=np.float64)).astype(np.float32)


def _erf(x):
    try:
        from scipy.special import erf

        return erf(x)
    except Exception:
        return np.vectorize(math.erf)(x).astype(x.dtype)


def _gather_att_core(y):
    """y_out [P, n_ycols] -> per-core att rows [ROWS_PER_CORE].
    Each row's dot = sum of its h=0 and h=1 half-dots (and the tail-split
    chunks of the last unit)."""
    units, _ = _unit_plan()
    att = np.zeros(ROWS_PER_CORE, dtype=np.float64)
    for unit in units:
        col = y[:, unit["ycols"]].sum(axis=1)
        r = unit["row0"]
        for p0, cnt in unit["spans"]:
            att[r : r + cnt] += col[p0 : p0 + cnt]
            r += cnt
    return att.astype(np.float32)


def _run_device(inputs, trace=False, **spmd_kwargs):
    """Run the dot-product phase on the 8 cores; return att [B, D] (pre-BN)
    and the BassKernelResults (for profiling from test harnesses)."""
    x = np.ascontiguousarray(np.asarray(inputs["x"], dtype=np.float32))
    w = _dct_weight(inputs["dw_w"])
    if PEW:
        w_name, w_val = "w_row", np.ascontiguousarray(w.reshape(1, N))
    else:
        w_name, w_val = "w_rep", np.ascontiguousarray(
            np.broadcast_to(w[None, :], (P, N))
        )

    nc = _build()
    b_per_core = B // N_CORES
    in_maps = []
    for c in range(N_CORES):
        xs = np.ascontiguousarray(
            x[c * b_per_core : (c + 1) * b_per_core].reshape(ROWS_PER_CORE, N)
        )
        in_maps.append({"x_sh": xs, w_name: w_val})

    res = bass_utils.run_bass_kernel_spmd(
        nc, in_maps, core_ids=list(range(N_CORES)), trace=trace, **spmd_kwargs
    )
    att = np.concatenate(
        [_gather_att_core(res.results[c]["y_out"]) for c in range(N_CORES)]
    ).reshape(B, D)
    return att, res


def _postprocess(att, inputs):
    """Host tail on the tiny [B, D] array: +dw_b, BatchNorm (global batch
    stats, training mode), exact GELU, 1x1 conv affine, softmax over D."""
    dw_b = np.float32(np.asarray(inputs["dw_b"]).reshape(-1)[0])
    gamma = np.float32(np.asarray(inputs["gamma"]).reshape(-1)[0])
    beta = np.float32(np.asarray(inputs["beta"]).reshape(-1)[0])
    conv_w = np.float32(np.asarray(inputs["conv_w"]).reshape(-1)[0])
    conv_b = np.float32(np.asarray(inputs["conv_b"]).reshape(-1)[0])

    att = att.astype(np.float32) + dw_b
    mean = att.mean(dtype=np.float64)
    var = np.mean((att.astype(np.float64) - mean) ** 2)
    inv_std = np.float32(1.0 / math.sqrt(var + BN_EPS))
    att = (att - np.float32(mean)) * inv_std * gamma + beta
    # Exact GELU: x * 0.5 * (1 + erf(x / sqrt(2)))
    att = (att * 0.5 * (1.0 + _erf(att / np.float32(math.sqrt(2.0))))).astype(
        np.float32
    )
    att1 = att * conv_w + conv_b
    att1 = att1 - att1.max(axis=-1, keepdims=True)
    e = np.exp(att1.astype(np.float32))
    att1 = (e / e.sum(axis=-1, keepdims=True)).astype(np.float32)
    att1 = att1[:, :, None]
    return att1, (np.float32(1.0) - att1).astype(np.float32)


def kernel(**inputs):
    att, _ = _run_device(inputs)
    return _postprocess(att, inputs)


# revision 4
# speedup vs baseline: 1.9550x; 1.9550x over previous
"""Trainium2 Bass kernel for nn_DctAtt (B=32, D=1024, N=4096, K=5).

The reference collapses to att[b,d] = x[b,d,:] . w  (w = C @ dw_w precomputed
on host), followed by tiny [32,1024] BN/GELU/softmax work done on host.
The device kernel streams x (512 MiB, data-parallel over B across 8 cores,
64 MiB/core) through fused DVE tensor_tensor_reduce dot products.

Trace findings this design is built on (ntff profiles):
  * A [128, 4096] f32 tile from a contiguous 2 MiB DRAM block is moved as
    128 contiguous 16 KiB descriptors, 8 per SDMA engine, and each engine
    sustains ~26.5 GB/s (aggregate ~425 GB/s, the practical per-core
    ceiling). Strided sources (column-sliced tiles -> 8 KiB lines) drop
    per-engine rate to ~17-20 GB/s -- never slice columns.
  * SDMA engine 15 intermittently runs at ~22 GB/s (known-slow engine).
    With uniform striping every dma_start gives each engine 8 lines, so
    the whole stream drains at the slowest engine's pace: the baseline is
    bimodal 181/221 us. Fix: "slim" tiles (120 rows) skip the 8
    partitions served by engine 15, so over the 33-tile schedule engine
    15 carries ~0.52x the per-engine line count and is never critical.
  * The old [128, 4096] w replica cost a 2 MiB HBM read serialized ahead
    of the x stream (~6 us). Now w is loaded as one 16 KiB row and
    broadcast across partitions on-chip with a K=1 TensorE matmul
    (ones[1,128]^T @ w[1,512]) through PSUM - zero SDMA work.

Unit plan (per core, 4096 rows): 33 tiles = 17 full (128 rows) + 16 slim
(120 rows), slim at odd tile indices; rows assigned to tiles sequentially
so every dma_start reads a contiguous DRAM block. y_sb[p, t] = row dot.
"""

import math

import numpy as np

import concourse.bacc as bacc
import concourse.mybir as mybir
import concourse.tile as tile
from concourse import bass_utils

# Problem constants (hardcoded: the grading harness ships only this file).
B, D, N = 32, 1024, 4096
K = 5
BN_EPS = 1e-5
N_CORES = 8
P = 128
ROWS_PER_CORE = (B // N_CORES) * D  # 4096

import os as _os

# In-flight [128, 4096] tiles (16 KiB/partition each).
XP_BUFS = int(_os.environ.get("DCT_BUFS", "8"))
# 1: de-weight SDMA engine 15 via slim tiles; 0: uniform 32 full tiles.
# NOTE: probe2 showed partial-partition dma_starts take a ~2x slower
# descriptor path (13 vs 26.9 GB/s per engine), so slim tiles as
# partial-partition transfers are a net loss; default off.
SLIM = int(_os.environ.get("DCT_SLIM", "0"))
# 1: broadcast w on-chip via PE (16 KiB HBM); 0: read [128, N] replica (2 MiB).
PEW = int(_os.environ.get("DCT_PEW", "1"))
# Partitions served by the slow SDMA engine (from probe.py); slim tiles
# skip exactly these.
EXCL = tuple(
    int(p) for p in _os.environ.get("DCT_EXCL", "92,93,94,95,124,125,126,127").split(",")
)


def _spans_excluding(excl):
    """Partition spans [p0, p1) covering 0..127 minus `excl`."""
    excl = set(excl)
    spans = []
    p = 0
    while p < P:
        if p in excl:
            p += 1
            continue
        q = p
        while q < P and q not in excl:
            q += 1
        spans.append((p, q - p))
        p = q
    return tuple(spans)


FULL_SPANS = ((0, P),)


def _unit_plan():
    """Static tile list shared by the device build and the host gather.

    tiles[i] = dict(row0, spans); rows are assigned sequentially, so each
    span's DMA reads a contiguous DRAM block of 16 KiB rows.
    """
    if SLIM:
        slim_spans = _spans_excluding(EXCL)
        tiles = []
        cur = 0
        for t in range(33):
            spans = slim_spans if (t % 2 == 1 and t < 32) else FULL_SPANS
            cap = sum(c for _, c in spans)
            tiles.append({"row0": cur, "spans": spans})
            cur += cap
        assert cur == ROWS_PER_CORE, cur
    else:
        tiles = [{"row0": 128 * t, "spans": FULL_SPANS} for t in range(32)]
    return tiles


_compiled_nc = None


def _build():
    """Build + compile the per-core Bass program (cached per process)."""
    global _compiled_nc
    if _compiled_nc is not None:
        return _compiled_nc

    tiles = _unit_plan()
    n_ycols = len(tiles)
    nc = bacc.Bacc(
        "TRN2",
        target_bir_lowering=False,
        debug=False,
        enable_asserts=False,
        num_devices=N_CORES,
    )
    f32 = mybir.dt.float32
    x_sh = nc.dram_tensor("x_sh", [ROWS_PER_CORE, N], f32, kind="ExternalInput").ap()
    if PEW:
        w_in = nc.dram_tensor("w_row", [1, N], f32, kind="ExternalInput").ap()
    else:
        w_in = nc.dram_tensor("w_rep", [P, N], f32, kind="ExternalInput").ap()
    y_out = nc.dram_tensor("y_out", [P, n_ycols], f32, kind="ExternalOutput").ap()

    with tile.TileContext(nc) as tc:
        with (
            tc.tile_pool(name="wp", bufs=1) as wp,
            tc.tile_pool(name="xp", bufs=XP_BUFS) as xp,
            tc.tile_pool(name="yp", bufs=1) as yp,
        ):
            w_sb = wp.tile([P, N], f32)
            if PEW:
                # w: one 16 KiB HBM read on the Scalar HWDGE ring, then
                # partition-broadcast through the PE (ones[1,128].T @ w[1,c])
                # and ACT copies PSUM -> SBUF. Zero SDMA cost vs the 2 MiB
                # replica read that used to head the x stream.
                w_row = wp.tile([1, N], f32)
                ones = wp.tile([1, P], f32)
                nc.scalar.dma_start(out=w_row, in_=w_in)
                nc.vector.memset(ones, 1.0)
                cw = 512  # one PSUM bank of f32
                with tc.tile_pool(name="ps", bufs=2, space="PSUM") as psp:
                    for c in range(N // cw):
                        ps = psp.tile([P, cw], f32)
                        nc.tensor.matmul(
                            ps,
                            ones,
                            w_row[:, c * cw : (c + 1) * cw],
                            start=True,
                            stop=True,
                        )
                        nc.scalar.copy(out=w_sb[:, c * cw : (c + 1) * cw], in_=ps)
            else:
                nc.scalar.dma_start(out=w_sb, in_=w_in)
            y_sb = yp.tile([P, n_ycols], f32)
            # Stride-0 free dim: the fused op's elementwise product is not
            # materialised (every element lands on the same column).
            dummy = wp.tile([P, 1], f32)
            for t, ut in enumerate(tiles):
                row0, spans = ut["row0"], ut["spans"]
                xt = xp.tile([P, N], f32)
                r = row0
                for p0, cnt in spans:
                    nc.sync.dma_start(
                        out=xt[p0 : p0 + cnt, :], in_=x_sh[r : r + cnt, :]
                    )
                    r += cnt
                # accum = sum(x * w) per partition = row dot. Slim tiles
                # leave the excluded partitions' stale contents in place;
                # they only pollute y rows the host never reads.
                nc.vector.affine_mul_reduce(
                    out=dummy.broadcast_to((P, N)),
                    accum_out=y_sb[:, t : t + 1],
                    in0=xt,
                    in1=w_sb,
                    scale=1.0,
                    bias=0.0,
                )
            nc.sync.dma_start(out=y_out, in_=y_sb)

    nc.compile()
    _compiled_nc = nc
    return nc


def _dct_weight(dw_w):
    """w = C @ dw_w in float64, where C is the [N, K] ortho DCT-II basis."""
    n = np.arange(N, dtype=np.float64)
    k = np.arange(K, dtype=np.float64)
    C = np.cos(np.pi * (2.0 * n[:, None] + 1.0) * k[None, :] / (2.0 * N))
    C *= math.sqrt(2.0 / N)
    C[:, 0] *= 1.0 / math.sqrt(2.0)
    return (C @ np.asarray(dw_w, dtype=np.float64)).astype(np.float32)


def _erf(x):
    try:
        from scipy.special import erf

        return erf(x)
    except Exception:
        return np.vectorize(math.erf)(x).astype(x.dtype)


def _gather_att_core(y):
    """y_out [P, n_ycols] -> per-core att rows [ROWS_PER_CORE]."""
    tiles = _unit_plan()
    att = np.empty(ROWS_PER_CORE, dtype=np.float32)
    for t, ut in enumerate(tiles):
        r = ut["row0"]
        for p0, cnt in ut["spans"]:
            att[r : r + cnt] = y[p0 : p0 + cnt, t]
            r += cnt
    return att


def _run_device(inputs, trace=False, **spmd_kwargs):
    """Run the dot-product phase on the 8 cores; return att [B, D] (pre-BN)
    and the BassKernelResults (for profiling from test harnesses)."""
    x = np.ascontiguousarray(np.asarray(inputs["x"], dtype=np.float32))
    w = _dct_weight(inputs["dw_w"])
    if PEW:
        w_name, w_val = "w_row", np.ascontiguousarray(w.reshape(1, N))
    else:
        w_name, w_val = "w_rep", np.ascontiguousarray(
            np.broadcast_to(w[None, :], (P, N))
        )

    nc = _build()
    b_per_core = B // N_CORES
    in_maps = []
    for c in range(N_CORES):
        xs = np.ascontiguousarray(
            x[c * b_per_core : (c + 1) * b_per_core].reshape(ROWS_PER_CORE, N)
        )
        in_maps.append({"x_sh": xs, w_name: w_val})

    res = bass_utils.run_bass_kernel_spmd(
        nc, in_maps, core_ids=list(range(N_CORES)), trace=trace, **spmd_kwargs
    )
    att = np.concatenate(
        [_gather_att_core(res.results[c]["y_out"]) for c in range(N_CORES)]
    ).reshape(B, D)
    return att, res


def _postprocess(att, inputs):
    """Host tail on the tiny [B, D] array: +dw_b, BatchNorm (global batch
    stats, training mode), exact GELU, 1x1 conv affine, softmax over D."""
    dw_b = np.float32(np.asarray(inputs["dw_b"]).reshape(-1)[0])
    gamma = np.float32(np.asarray(inputs["gamma"]).reshape(-1)[0])
    beta = np.float32(np.asarray(inputs["beta"]).reshape(-1)[0])
    conv_w = np.float32(np.asarray(inputs["conv_w"]).reshape(-1)[0])
    conv_b = np.float32(np.asarray(inputs["conv_b"]).reshape(-1)[0])

    att = att.astype(np.float32) + dw_b
    mean = att.mean(dtype=np.float64)
    var = np.mean((att.astype(np.float64) - mean) ** 2)
    inv_std = np.float32(1.0 / math.sqrt(var + BN_EPS))
    att = (att - np.float32(mean)) * inv_std * gamma + beta
    # Exact GELU: x * 0.5 * (1 + erf(x / sqrt(2)))
    att = (att * 0.5 * (1.0 + _erf(att / np.float32(math.sqrt(2.0))))).astype(
        np.float32
    )
    att1 = att * conv_w + conv_b
    att1 = att1 - att1.max(axis=-1, keepdims=True)
    e = np.exp(att1.astype(np.float32))
    att1 = (e / e.sum(axis=-1, keepdims=True)).astype(np.float32)
    att1 = att1[:, :, None]
    return att1, (np.float32(1.0) - att1).astype(np.float32)


def kernel(**inputs):
    att, _ = _run_device(inputs)
    return _postprocess(att, inputs)
